# revision 7
# baseline (speedup 1.0000x reference)
"""Trainium2 Bass kernel for a Neural-CA model (nn_CAModel).

Per step (16 steps):
  perception = [x, sobel_x(x), sobel_y(x)]   (depthwise 3x3, SAME)
  h    = relu(w1 @ perception + b1)          (1x1 conv, 48 -> 128)
  diff = w2 @ h + b2                         (1x1 conv, 128 -> 16)
  new  = x + diff * mask                     (mask: per-pixel bernoulli(0.5))
  x    = new * (maxpool3x3(new[:,3]) > 0.1)  (alive gate)

Strategy: pure data parallel, one batch image per NeuronCore (B=8 = n_cores).
The perception+w1 stage folds into one 3x3 (16->128) conv evaluated as 9
accumulating PE matmuls whose rhs are free-dim-shifted views of the
zero-padded state in SBUF.  float32r matmuls (full PE rate at N>=256).
Stochastic masks are precomputed host-side (jax threefry is bit-exact) and
streamed from HBM.
"""

import sys

sys.path.insert(0, "/opt/trn_rl_repo")

import numpy as np

B, C, H, W = 8, 16, 128, 128
HID = 128
Hp, Wp = H + 2, W + 2          # zero-padded image
NPIX = H * W                   # 16384
PADN = Hp * Wp                 # 16900
TILE = 512                     # pixels per PE tile (4 image rows)
RPT = TILE // W                # rows per tile = 4
NT = NPIX // TILE              # 32

_COMPILED = {}
_LAST_IN_MAPS = None


def _fold_conv_weights(w1):
    """k[o,c,dy,dx] for the fused perception+w1 3x3 conv."""
    sy = np.array([[1.0, 2.0, 1.0], [0.0, 0.0, 0.0], [-1.0, -2.0, -1.0]],
                  np.float32) / 8.0
    sx = sy.T.copy()
    k = np.zeros((HID, C, 3, 3), np.float32)
    k[:, :, 1, 1] += w1[:, 0:C]
    for dy in (-1, 0, 1):
        for dx in (-1, 0, 1):
            k[:, :, dy + 1, dx + 1] += w1[:, C:2 * C] * sx[dy + 1, dx + 1]
            k[:, :, dy + 1, dx + 1] += w1[:, 2 * C:3 * C] * sy[dy + 1, dx + 1]
    return k


def _masks_np(n_steps):
    import jax

    cpu = jax.devices("cpu")[0]
    with jax.default_device(cpu):
        keys = jax.random.split(jax.random.key(42), n_steps)
        out = []
        for s in range(n_steps):
            u = jax.random.uniform(keys[s], (B, 1, H, W))
            out.append(np.asarray(u) < 0.5)
    return np.stack(out)  # [S, B, 1, H, W] bool


def _win(ap_full, offset, dims):
    """AP over the same partitions with explicit free dims [[step, count]..]."""
    import bass_rust
    part = ap_full.ap[0]
    return type(ap_full)(ap_full.tensor, ap_full.offset + offset,
                         [list(part)] + [list(d) for d in dims])


def _build_program(n_steps):
    import concourse.bass as bass
    import concourse.mybir as mybir
    import concourse.tile as tile
    from concourse import bacc

    f32 = mybir.dt.float32
    f32r = mybir.dt.float32r
    Alu = mybir.AluOpType
    Act = mybir.ActivationFunctionType

    nc = bacc.Bacc("TRN2", target_bir_lowering=False, debug=False, num_devices=B)

    s0_d = nc.dram_tensor("s0", [C, PADN], f32r, kind="ExternalInput")
    mask_d = nc.dram_tensor("mask16", [n_steps, C, NPIX], f32, kind="ExternalInput")
    taps_d = nc.dram_tensor("taps", [C, 9 * HID], f32r, kind="ExternalInput")
    w2t_d = nc.dram_tensor("w2T", [HID, C], f32r, kind="ExternalInput")
    b2_d = nc.dram_tensor("b2row", [1, C], f32r, kind="ExternalInput")
    b1_d = nc.dram_tensor("b1col", [HID, 1], f32, kind="ExternalInput")
    ones_d = nc.dram_tensor("ones", [1, TILE], f32r, kind="ExternalInput")
    y_d = nc.dram_tensor("y", [C, NPIX], f32, kind="ExternalOutput")

    with tile.TileContext(nc) as tc:
        with (
            tc.tile_pool(name="persist", bufs=1) as pp,
            tc.tile_pool(name="hm", bufs=3) as hmp,
            tc.tile_pool(name="small", bufs=4) as sp,
            tc.tile_pool(name="hps", bufs=2, space="PSUM") as hpsp,
            tc.tile_pool(name="dps", bufs=2, space="PSUM") as dpsp,
        ):
            S = pp.tile([C, PADN], f32r)
            newf = pp.tile([C, NPIX], f32)
            alivef = pp.tile([C, NPIX], mybir.dt.bfloat16)
            alpha_sh = pp.tile([128, 3 * Wp], f32)
            pooledH = pp.tile([128, Wp], f32)
            aliveH = pp.tile([128, W], f32)
            taps = pp.tile([C, 9 * HID], f32r)
            w2t = pp.tile([HID, C], f32r)
            b2r = pp.tile([1, C], f32r)
            b1c = pp.tile([HID, 1], f32)
            ones = pp.tile([1, TILE], f32r)

            # ---- init ----
            nc.vector.memset(alpha_sh[:], 0.0)
            nc.sync.dma_start(S[:], s0_d[:])  # host sends fully padded state
            nc.sync.dma_start(taps[:], taps_d[:])
            nc.sync.dma_start(w2t[:], w2t_d[:])
            nc.sync.dma_start(b2r[:], b2_d[:])
            nc.sync.dma_start(b1c[:], b1_d[:])
            nc.sync.dma_start(ones[:], ones_d[:])

            tap_off = [dy * Wp + dx for dy in (-1, 0, 1) for dx in (-1, 0, 1)]
            PB = Wp + 1  # interior (0,0) flat offset

            for step in range(n_steps):
                for t in range(NT):
                    base = PB + t * RPT * Wp
                    h_ps = hpsp.tile([HID, TILE], f32)
                    for i in range(9):
                        rhs = _win(S[:], base + tap_off[i], [[Wp, RPT], [1, W]])
                        nc.tensor.matmul(
                            h_ps[:],
                            taps[:, i * HID:(i + 1) * HID],
                            rhs,
                            start=(i == 0),
                            stop=(i == 8),
                        )
                    hm = hmp.tile([HID, TILE], f32r)
                    nc.scalar.activation(hm[:], h_ps[:], Act.Relu, bias=b1c[:, :])
                    d_ps = dpsp.tile([C, TILE], f32)
                    nc.tensor.matmul(d_ps[:], w2t[:],
                                     hm[:], start=True, stop=False)
                    nc.tensor.matmul(d_ps[:], b2r[:],
                                     ones[:], start=False, stop=True)
                    mk = sp.tile([C, TILE], f32)
                    nc.sync.dma_start(mk[:], mask_d[step, :, t * TILE:(t + 1) * TILE])
                    u = sp.tile([C, TILE], f32)
                    nc.vector.tensor_mul(out=u[:], in0=d_ps[:], in1=mk[:])
                    # new = u + S_interior
                    sint = _win(S[:], base, [[Wp, RPT], [1, W]])
                    nout = _win(newf[:], t * TILE, [[W, RPT], [1, W]])
                    uin = _win(u[:], 0, [[W, RPT], [1, W]])
                    nc.vector.tensor_add(out=nout, in0=uin, in1=sint)

                # ---- alive gate ----
                nfv = newf[:].rearrange("c (h w) -> c h w", w=W)
                asv = alpha_sh[:].rearrange("p (k w) -> p k w", w=Wp)
                # strip k=0: alpha rows h-1 -> partitions 1..127
                nc.sync.dma_start(asv[1:128, 0, 1:1 + W], nfv[3:4, 0:H - 1, :])
                # strip k=1: alpha rows h
                nc.sync.dma_start(asv[0:128, 1, 1:1 + W], nfv[3:4, :, :])
                # strip k=2: alpha rows h+1 -> partitions 0..126
                nc.sync.dma_start(asv[0:127, 2, 1:1 + W], nfv[3:4, 1:H, :])
                # H-direction max over the 3 strips
                nc.vector.tensor_max(out=pooledH[:], in0=alpha_sh[:, 0:Wp],
                                     in1=alpha_sh[:, Wp:2 * Wp])
                nc.vector.tensor_max(out=pooledH[:], in0=pooledH[:],
                                     in1=alpha_sh[:, 2 * Wp:3 * Wp])
                # W-direction max, overlapping windows of 3
                nc.vector.tensor_max(out=aliveH[:], in0=pooledH[:, 0:W],
                                     in1=pooledH[:, 1:1 + W])
                nc.vector.tensor_max(out=aliveH[:], in0=aliveH[:],
                                     in1=pooledH[:, 2:2 + W])
                nc.vector.tensor_scalar(
                    out=aliveH[:], in0=aliveH[:], scalar1=0.1, scalar2=None,
                    op0=Alu.is_gt,
                )
                for ch in range(C):
                    nc.gpsimd.dma_start(alivef[ch:ch + 1, :], aliveH[:, :])
                # S_interior = newf * alivef  (next state, in place)
                nc.vector.tensor_mul(
                    out=_win(S[:], PB, [[Wp, H], [1, W]]),
                    in0=_win(newf[:], 0, [[W, H], [1, W]]),
                    in1=_win(alivef[:], 0, [[W, H], [1, W]]),
                )

            nc.sync.dma_start(y_d[:], _win(S[:], PB, [[Wp, H], [1, W]]).bitcast(f32))

    nc.compile()
    return nc


def kernel(x, w1, b1, w2, b2, n_steps):
    from concourse.bass_utils import run_bass_kernel_spmd

    n_steps = int(n_steps)
    x = np.asarray(x, np.float32)
    w1 = np.asarray(w1, np.float32)
    b1 = np.asarray(b1, np.float32)
    w2 = np.asarray(w2, np.float32)
    b2 = np.asarray(b2, np.float32)

    if n_steps not in _COMPILED:
        _COMPILED[n_steps] = _build_program(n_steps)
    nc = _COMPILED[n_steps]

    k = _fold_conv_weights(w1)           # [HID, C, 3, 3]
    taps = np.zeros((C, 9 * HID), np.float32)
    i = 0
    for dy in (-1, 0, 1):
        for dx in (-1, 0, 1):
            taps[:, i * HID:(i + 1) * HID] = k[:, :, dy + 1, dx + 1].T
            i += 1

    masks = _masks_np(n_steps)           # [S, B, 1, H, W] bool

    in_maps = []
    for b in range(B):
        s0 = np.zeros((C, Hp, Wp), np.float32)
        s0[:, 1:1 + H, 1:1 + W] = x[b]
        m16 = np.broadcast_to(
            masks[:, b, 0].reshape(n_steps, 1, NPIX), (n_steps, C, NPIX)
        ).astype(np.float32)
        in_maps.append({
            "s0": s0.reshape(C, PADN),
            "mask16": np.ascontiguousarray(m16),
            "taps": taps,
            "w2T": np.ascontiguousarray(w2.T),
            "b2row": b2.reshape(1, C).copy(),
            "b1col": b1.reshape(HID, 1).copy(),
            "ones": np.ones((1, TILE), np.float32),
        })

    global _LAST_IN_MAPS
    _LAST_IN_MAPS = in_maps
    res = run_bass_kernel_spmd(nc, in_maps, core_ids=list(range(B)))
    out = np.stack([res.results[b]["y"].reshape(C, H, W) for b in range(B)])
    return out
